# revision 5
# baseline (speedup 1.0000x reference)
"""Trainium2 Bass kernel for nn_MessagePassingLayer (gnn_message_passing), v2.

Computes, for x:[B,C,N,1] f32, edge_index:[B,N,K] i32, alpha scalar:
    out[b,c,n] = x[b,c,n]*(1+alpha) + sum_k x[b,c,edge_index[b,n,k]]

Sharding: B=8 batch samples, one per NeuronCore (data parallel).

Mechanism: the gather+sum is a dense fp8 matmul of streamed Aᵀ
(adjacency counts) against a stationary holding TWO fp8 copies of x
(hi + residual), recovering ~bf16 precision in one pass.  Changes vs
the 68.4us baseline (measured best ~61.2us; runs are +/-15% with HBM
contention on the shared device):
  - (1+alpha) is folded into Aᵀ's DIAGONAL on the host (exact in e4m3
    when 1+alpha is an integer <= 16 - counts; generic alpha path
    asserts representability).  The x/alpha loads and the
    scalar_tensor_tensor fold disappear; the per-group fold is an ACT
    copy of psum[64:128] to SBUF + a DVE add with psum[0:64] (DVE may
    only read one PSUM operand per instruction).
  - dst is split into 4 quarters (was 2 halves) so the post-stream
    fold+store tail only covers 1024 columns; the last quarter stores
    per 512-col group so just a 65 KB store trails the final fold.
  - Aᵀ streams as 14 chunks: small leading chunks (128/256 KB) start
    the rings ~2us earlier, 2 MB mid-stream chunks cut Q7 desc-gen
    work and semaphore-teardown cost, small trailing chunks cut the
    final matmul lag.  One SBUF tile per chunk -- slices of a shared
    tile create false deps in the tile tracker that stall desc-gen
    ~1.5us at every quarter boundary (measured).
  - Ring assignment is (SWDGE instruction index) % 4, which matches
    DMASW sem-lane locking exactly; queue names patched post-compile.
  - Stores stay on SWDGE: HWDGE measured ~16 GB/s when the SWDGE
    stream saturates the 16 SDMA engines (packet round-robin starves
    it), so offloading any Aᵀ chunk or store there LOSES time.
  - Stream sustains ~380 GB/s (fabric 435, HBM-per-NC ~358-390); the
    16.78 MB Aᵀ is irreducible for a PE-based gather (fp8 is the
    narrowest moving dtype; GPSIMD ap_gather/DMA row-gather are
    descriptor-rate-bound ~130+ us/core and cannot beat it).
"""
import os
import sys
import types

import numpy as np

B, C, N, K = 8, 64, 4096, 16
NCORES = 8
P = 128
NPAIR = N // (2 * P)     # 16 contraction pair-blocks (DoubleRow: 256 rows)
GRP = 512                # psum bank free size (f32)
# dst phases (col0, width): wide early phases amortize LDWEIGHTS (one
# stationary load serves all groups of a phase per v); the 512-col
# final phase shrinks the post-stream tail to one matmul + one fold +
# one 65 KB store. Groups (width/512) sum to the 8 PSUM banks: 4+3+1.
_PH = int(os.environ.get("KERNEL_PHASEPLAN", "3"))
if _PH == 2:
    PHASES = [(0, 3584), (3584, 512)]
elif _PH == 4:
    PHASES = [(0, 2048), (2048, 1536), (3584, 256), (3840, 256)]
else:
    PHASES = [(0, 2048), (2048, 1536), (3584, 512)]

# Aᵀ chunk plan: (phase, v0, nv) in issue/consumption order. Small
# leading chunks let the rings start transferring while desc-gen for
# the rest proceeds; small final chunks trim the tail matmul lag.
def _spans(sizes):
    out, v0 = [], 0
    for s in sizes:
        out.append((v0, s)); v0 += s
    return out

if _PH == 2:
    _QSPANS = {
        0: _spans([1, 1, 2, 2, 2, 2, 2, 2, 2]),
        1: _spans([4, 4, 4, 2, 1, 1]),
    }
elif _PH == 4:
    _QSPANS = {
        0: _spans([1, 1, 2, 4, 4, 4]),   # 512 KB per v-unit
        1: _spans([4, 4, 4, 4]),         # 384 KB per v-unit
        2: _spans([8, 8]),               # 64 KB per v-unit
        3: _spans([8, 4, 2, 1, 1]),      # 64 KB per v-unit
    }
else:
    _QSPANS = {
        0: _spans([1, 1, 2, 4, 4, 4]),   # 512 KB per v-unit
        1: _spans([4, 4, 4, 4]),         # 384 KB per v-unit
        2: _spans([4, 4, 4, 2, 1, 1]),   # 128 KB per v-unit
    }
CHUNKS = [(ph, v0, nv) for ph in range(len(PHASES))
          for (v0, nv) in _QSPANS[ph]]

LAST_EXEC_NS = None


# ---------------------------------------------------------------------------
# axon NTFF profile hook shim (the agent image's antenv lacks axon_hooks)
# ---------------------------------------------------------------------------
def _install_profile_shim():
    if "antenv.axon_hooks" in sys.modules:
        return
    try:
        import antenv

        mod = types.ModuleType("antenv.axon_hooks")
        mod._hook = None
        mod.set_axon_ntff_profile_hook = lambda h: setattr(mod, "_hook", h)
        mod.get_axon_ntff_profile_hook = lambda: mod._hook
        sys.modules["antenv.axon_hooks"] = mod
        antenv.axon_hooks = mod
        from trn_agent_boot.trn_boot import _ntff_profile_via_ctypes

        mod.set_axon_ntff_profile_hook(
            _ntff_profile_via_ctypes("/opt/axon/libaxon_pjrt.so")
        )
    except Exception:
        pass


# ---------------------------------------------------------------------------
# Walrus in this container rejects >1 sync-wait per instruction. Split any
# multi-wait instruction into single-wait NoOps on the same engine.
# ---------------------------------------------------------------------------
def _split_multiwaits(nc, mybir):
    cnt = [0]
    for f in nc.m.functions:
        for bb in f.blocks:
            new_list = []
            for ins in bb.instructions:
                si = ins.sync_info
                if si is not None and si.on_wait and len(si.on_wait) > 1:
                    waits = list(si.on_wait)
                    for w in waits[:-1]:
                        cnt[0] += 1
                        nop = mybir.InstNoOp(name=f"I-waitsplit-{cnt[0]}")
                        nop.engine = ins.engine
                        nop.sync_info = mybir.SyncInfo(on_wait=[w], on_update=[])
                        try:
                            nc.register_instruction(nop, overwrite=True)
                        except Exception:
                            pass
                        new_list.append(nop)
                    ins.sync_info = mybir.SyncInfo(
                        on_wait=[waits[-1]], on_update=list(si.on_update)
                    )
                new_list.append(ins)
            bb.instructions = new_list


# ---------------------------------------------------------------------------
# Legalization splits every InstMatmult into Ldweights+Matmult, reloading
# the stationary even when consecutive matmuls share it. Drop the redundant
# reloads (PE array state persists); preserve any sync via a PE NoOp.
# ---------------------------------------------------------------------------
def _dedup_ldweights(nc, mybir):
    dropped = [0]
    for f in nc.m.functions:
        for bb in f.blocks:
            last_sig = None
            new_list = []
            for ins in bb.instructions:
                if isinstance(ins, mybir.InstLdweights):
                    sig = repr(ins.ins[0])
                    if sig == last_sig:
                        si = ins.sync_info
                        dropped[0] += 1
                        if si is not None and (si.on_wait or si.on_update):
                            nop = mybir.InstNoOp(
                                name=f"I-ldwdedup-{dropped[0]}")
                            nop.engine = ins.engine
                            nop.sync_info = si
                            try:
                                nc.register_instruction(nop, overwrite=True)
                            except Exception:
                                pass
                            new_list.append(nop)
                        continue
                    last_sig = sig
                elif isinstance(ins, mybir.InstMatmult):
                    pass  # uses the loaded array, does not clobber it
                elif getattr(ins, "engine", None) == mybir.EngineType.PE:
                    last_sig = None  # unknown PE instruction: be safe
                new_list.append(ins)
            bb.instructions = new_list
    return dropped[0]


# ---------------------------------------------------------------------------
# Device program
# ---------------------------------------------------------------------------
SWI = bool(int(os.environ.get("KERNEL_SWI", "1")))  # DoubleRowSwInterleave
NWARM = int(os.environ.get("KERNEL_NWARM", "20"))


def _build_program():
    import concourse.mybir as mybir
    import concourse.tile as tile
    from concourse import bacc

    nc = bacc.Bacc("TRN2", target_bir_lowering=False, debug=False,
                   num_devices=NCORES, num_swdge_queues=4,
                   dynamic_dma_scratch_size=32768)
    # Aᵀ fp8 byte chunks.  Chunk (ph, v0, nv) with phase (col0, width)
    # holds, p-major:
    #   at_c[p, ((v-v0)*2 + t)*width + n] = A[256*v + 128*t + p, col0 + n]
    # (A includes the (1+alpha) diagonal).
    at_d = [
        nc.dram_tensor(f"at{c}", [P, nv * 2 * PHASES[ph][1]],
                       mybir.dt.uint8, kind="ExternalInput")
        for c, (ph, v0, nv) in enumerate(CHUNKS)
    ]
    # stationary x fp8 bytes: [p, (q, t, s, c)] with s = {hi, w}
    xs_d = nc.dram_tensor("xs", [P, NPAIR * 2 * 2 * C], mybir.dt.uint8,
                          kind="ExternalInput")
    out_d = nc.dram_tensor("out", [C, N], mybir.dt.bfloat16,
                           kind="ExternalOutput")

    fp8 = mybir.dt.float8e4
    ring_plan = {}

    with tile.TileContext(nc) as tc:
        with tc.tile_pool(name="sbuf", bufs=1) as pool, \
             tc.tile_pool(name="psum", bufs=1, space="PSUM") as ppool:
            xs_sb = pool.tile([P, NPAIR * 2 * 2 * C], mybir.dt.uint8,
                              tag="xs")
            o_sb = pool.tile([C, N], mybir.dt.bfloat16, tag="o")
            GW = []  # per-bank group width
            for (col0, width) in PHASES:
                gw = min(GRP, width)
                GW += [gw] * (width // gw)
            NBANK = len(GW)
            t_sb = [pool.tile([C, GW[g]], mybir.dt.bfloat16, tag=f"t{g}",
                              name=f"t{g}") for g in range(NBANK)]
            ps = [ppool.tile([P, GW[g]], mybir.dt.float32, tag=f"ps{g}",
                             name=f"ps{g}") for g in range(NBANK)]
            # one tile per chunk: slices of a shared tile would create
            # false write-after-read deps in the tile tracker and stall
            # the Q7 desc-gen at phase boundaries
            at_sb = [pool.tile([P, nv * 2 * PHASES[ph][1]], mybir.dt.uint8,
                               tag=f"at{c}", name=f"at{c}")
                     for c, (ph, v0, nv) in enumerate(CHUNKS)]
            # map (phase, v) -> (chunk tile, local v)
            v_map = {}
            for c, (ph, v0, nv) in enumerate(CHUNKS):
                for dv in range(nv):
                    v_map[(ph, v0 + dv)] = (c, dv)

            def swdge(inst, ring):
                ring_plan[inst.ins.name] = ring
                return inst

            # xs first (gates the warmup matmuls), then the Aᵀ chunks.
            # Ring = (SWDGE instruction index) % 4, which matches the
            # DMASW lane-locking exactly (lane = idx % 8, lane -> ring
            # idx % 4), so the post-compile patch never has to override.
            sw_idx = [0]

            def swdge_auto(inst):
                ring_plan[inst.ins.name] = sw_idx[0] % 4
                sw_idx[0] += 1
                return inst

            # NOTE: offloading chunks/stores to the HWDGE rings was tried
            # and measured ~16 GB/s while the SWDGE stream saturates the
            # 16 shared SDMA engines -- never do it.
            swdge_auto(nc.gpsimd.dma_start(out=xs_sb[:], in_=xs_d.ap()))
            for c, (ph, v0, nv) in enumerate(CHUNKS):
                swdge_auto(nc.gpsimd.dma_start(
                    out=at_sb[c][:], in_=at_d[c].ap()))

            # stationary views: [128, 2, 128] fp8 per pair v (hi|w columns)
            xs3 = xs_sb[:].rearrange("p (v t sc) -> p v t sc",
                                     v=NPAIR, t=2)

            # PE p-state warm-up: garbage matmuls (into ps[0], discarded by
            # the first real start=True matmul) so the tensor engine is at
            # full clock when the Aᵀ stream lands.
            wmv = xs_sb[:].rearrange("p (t n) -> p t n", t=2).bitcast(fp8)
            for i in range(NWARM):
                nc.tensor.matmul(
                    ps[0][:],
                    xs3[:, 0, :, :].bitcast(fp8),
                    wmv[:, :, 0:GRP],
                    start=(i == 0),
                    stop=(i == NWARM - 1),
                    perf_mode=(
                        mybir.MatmulPerfMode.DoubleRowSwInterleave
                        if SWI else mybir.MatmulPerfMode.DoubleRow),
                    skip_group_check=True,
                )

            PM = (mybir.MatmulPerfMode.DoubleRowSwInterleave if SWI
                  else mybir.MatmulPerfMode.DoubleRow)
            bank0 = 0
            for ph, (col0, width) in enumerate(PHASES):
                gw = min(GRP, width)
                ngrp = width // gw
                for v in range(NPAIR):
                    c, dv = v_map[(ph, v)]
                    nvc = CHUNKS[c][2]
                    at4 = at_sb[c][:].rearrange("p (v t n) -> p v t n",
                                                v=nvc, t=2).bitcast(fp8)
                    lhsT = xs3[:, v, :, :].bitcast(fp8)
                    for g in range(ngrp):
                        mv = at4[:, dv, :, g * gw:(g + 1) * gw]
                        nc.tensor.matmul(
                            ps[bank0 + g][:], lhsT, mv,
                            start=(v == 0),
                            stop=(v == NPAIR - 1),
                            perf_mode=PM,
                        )
                # phase ph done: fold hi+w partitions, store on the SWDGE
                # rings (HWDGE measured a pathetic ~16 GB/s). DVE can only
                # read ONE input from PSUM, so ACT copies the w rows to
                # SBUF (pipelines with DVE across groups). One store per
                # phase; the last phase is a single 512-col group so just
                # a 65 KB store (+receipt) sits after the final fold.
                for g in range(ngrp):
                    gi = bank0 + g
                    lo = col0 + g * gw
                    nc.scalar.copy(out=t_sb[gi][:], in_=ps[gi][C:2 * C, :])
                    nc.vector.tensor_add(
                        out=o_sb[:, lo:lo + gw],
                        in0=ps[gi][0:C, :],
                        in1=t_sb[gi][:],
                    )
                swdge_auto(nc.gpsimd.dma_start(
                    out=out_d.ap()[:, col0:col0 + width],
                    in_=o_sb[:, col0:col0 + width],
                ))
                bank0 += ngrp

    nc.compile()
    # Spread SWDGE dma_starts across the 4 queue rings (see baseline notes:
    # DMASW sem lanes are handed out round-robin over the final instruction
    # order and each lane locks to one ring, so walk the compiled order and
    # keep lanes consistent, preferring the planned ring at each lane's
    # first use).
    lane_ring = {}
    idx = 0
    for f in nc.m.functions:
        for bb in f.blocks:
            for ins in bb.instructions:
                if ins.name not in ring_plan:
                    continue
                lane = idx % 8
                idx += 1
                ring = lane_ring.setdefault(lane, ring_plan[ins.name])
                if ring:
                    ins.queue = f"qPoolDynamic{ring}"
    if bool(int(os.environ.get("KERNEL_DEDUP_LDW", "1"))):
        _dedup_ldweights(nc, mybir)
    _split_multiwaits(nc, mybir)
    return nc


_PROGRAM = None
_PROGRAM_ALPHA = None


def _get_program(alpha_v):
    global _PROGRAM, _PROGRAM_ALPHA
    if _PROGRAM is None:
        _PROGRAM = _build_program()
        _PROGRAM_ALPHA = alpha_v
    return _PROGRAM


# ---------------------------------------------------------------------------
# Host glue
# ---------------------------------------------------------------------------
def _fp8_lut():
    import ml_dtypes

    return np.arange(K + 1).astype(ml_dtypes.float8_e4m3fn).view(np.uint8)


_LUT = None


def _prep_at_chunks(edge_b, alpha_v):
    """edge_b [N, K] int32 -> list of Aᵀ fp8 chunk arrays (device layout).

    Aᵀ[src, dst] = |{k: edge[dst, k] == src}| + (1+alpha)*[src == dst].
    For integer (1+alpha) with max entry <= 16 this is exact in e4m3 via
    the count LUT.
    """
    global _LUT
    if _LUT is None:
        _LUT = _fp8_lut()
    src = edge_b.astype(np.int64)                       # [N dst, K]
    flat = (src * N + np.arange(N, dtype=np.int64)[:, None]).ravel()
    cnt = np.bincount(flat, minlength=N * N)            # Aᵀ[src, dst] counts
    a1 = float(1.0 + alpha_v)
    diag = np.arange(N, dtype=np.int64) * (N + 1)
    if a1 == round(a1) and 0 <= a1 <= 16:
        cnt[diag] += int(round(a1))
        assert cnt[diag].max() <= K, "diagonal count overflow for e4m3"
        at = _LUT[cnt]                                  # uint8 fp8 bytes
    else:
        import ml_dtypes

        vals = cnt.astype(np.float32)
        vals[diag] += np.float32(a1)
        q = vals.astype(ml_dtypes.float8_e4m3fn)
        assert np.array_equal(q.astype(np.float32), vals), \
            "1+alpha not exactly representable in e4m3"
        at = q.view(np.uint8)
    at = at.reshape(N, N)
    chunks = []
    for (ph, v0, nv) in CHUNKS:
        col0, width = PHASES[ph]
        # rows 256v+128t+p for v in [v0, v0+nv), cols col0 + n
        blk = at[256 * v0:256 * (v0 + nv), col0:col0 + width]
        b4 = blk.reshape(nv, 2, P, width)               # (v, t, p, n)
        b4 = b4.transpose(2, 0, 1, 3)                   # (p, v, t, n)
        chunks.append(np.ascontiguousarray(
            b4.reshape(P, nv * 2 * width)))
    return chunks


def _prep_xs(xt_b):
    """xt_b [N, C] f32 node-major -> stationary fp8 bytes [128, v*t*s*C]."""
    import ml_dtypes

    hi = xt_b.astype(ml_dtypes.float8_e4m3fn)
    w = (xt_b - hi.astype(np.float32)).astype(ml_dtypes.float8_e4m3fn)
    hw = np.stack([hi.view(np.uint8), w.view(np.uint8)], axis=1)  # [N, s, C]
    hw = hw.reshape(NPAIR, 2, P, 2, C)                  # (v, t, p, s, c)
    hw = hw.transpose(2, 0, 1, 3, 4)                    # (p, v, t, s, c)
    out = np.ascontiguousarray(hw.reshape(P, NPAIR * 2 * 2 * C))
    if SWI:
        # DoubleRowSwInterleave weight layout: flat cols per pair v are
        # [A127 B127 A126 B126 ... A0 B0] where A/B are the two k-tiles
        # and the logical column index is reversed.
        w4 = out.reshape(P, NPAIR, 2, 2 * C)            # (p, v, t, m)
        wr = w4[:, :, :, ::-1]                          # m -> reversed j
        wi = np.transpose(wr, (0, 1, 3, 2))             # (p, v, j, t)
        out = np.ascontiguousarray(wi.reshape(P, NPAIR * 2 * 2 * C))
    return out


def kernel(x, edge_index, alpha):
    global LAST_EXEC_NS
    _install_profile_shim()
    from concourse import bass_utils

    x = np.asarray(x)
    edge_index = np.asarray(edge_index)
    alpha_v = np.float32(np.asarray(alpha))

    nc = _get_program(float(alpha_v))

    in_maps = []
    for b in range(B):
        xt = np.ascontiguousarray(x[b, :, :, 0].T)      # [N, C]
        m = {"xs": _prep_xs(xt)}
        for c, arr in enumerate(_prep_at_chunks(edge_index[b], alpha_v)):
            m[f"at{c}"] = arr
        in_maps.append(m)

    trace = bool(int(os.environ.get("KERNEL_PROFILE", "0")))
    res = bass_utils.run_bass_kernel_spmd(
        nc, in_maps, core_ids=list(range(NCORES)), trace=trace
    )
    LAST_EXEC_NS = res.exec_time_ns

    out = np.empty((B, C, N, 1), dtype=np.float32)
    for b in range(B):
        out[b, :, :, 0] = res.results[b]["out"].astype(np.float32)
    return out


# revision 6
# speedup vs baseline: 1.1321x; 1.1321x over previous
"""Trainium2 Bass kernel for nn_MessagePassingLayer (gnn_message_passing), v2.

Computes, for x:[B,C,N,1] f32, edge_index:[B,N,K] i32, alpha scalar:
    out[b,c,n] = x[b,c,n]*(1+alpha) + sum_k x[b,c,edge_index[b,n,k]]

Sharding: B=8 batch samples, one per NeuronCore (data parallel).

Mechanism: the gather+sum is a dense fp8 matmul of streamed Aᵀ
(adjacency counts) against a stationary holding TWO fp8 copies of x
(hi + residual), recovering ~bf16 precision in one pass.  Changes vs
the 68.4us baseline (measured best ~61.2us; runs are +/-15% with HBM
contention on the shared device):
  - (1+alpha) is folded into Aᵀ's DIAGONAL on the host (exact in e4m3
    when 1+alpha is an integer <= 16 - counts; generic alpha path
    asserts representability).  The x/alpha loads and the
    scalar_tensor_tensor fold disappear; the per-group fold is an ACT
    copy of psum[64:128] to SBUF + a DVE add with psum[0:64] (DVE may
    only read one PSUM operand per instruction).
  - dst is split into 4 quarters (was 2 halves) so the post-stream
    fold+store tail only covers 1024 columns; the last quarter stores
    per 512-col group so just a 65 KB store trails the final fold.
  - Aᵀ streams as 14 chunks: small leading chunks (128/256 KB) start
    the rings ~2us earlier, 2 MB mid-stream chunks cut Q7 desc-gen
    work and semaphore-teardown cost, small trailing chunks cut the
    final matmul lag.  One SBUF tile per chunk -- slices of a shared
    tile create false deps in the tile tracker that stall desc-gen
    ~1.5us at every quarter boundary (measured).
  - Ring assignment is (SWDGE instruction index) % 4, which matches
    DMASW sem-lane locking exactly; queue names patched post-compile.
  - Stores stay on SWDGE: HWDGE measured ~16 GB/s when the SWDGE
    stream saturates the 16 SDMA engines (packet round-robin starves
    it), so offloading any Aᵀ chunk or store there LOSES time.
  - Stream sustains ~380 GB/s (fabric 435, HBM-per-NC ~358-390); the
    16.78 MB Aᵀ is irreducible for a PE-based gather (fp8 is the
    narrowest moving dtype; GPSIMD ap_gather/DMA row-gather are
    descriptor-rate-bound ~130+ us/core and cannot beat it).
"""
import os
import sys
import types

import numpy as np

B, C, N, K = 8, 64, 4096, 16
NCORES = 8
P = 128
NPAIR = N // (2 * P)     # 16 contraction pair-blocks (DoubleRow: 256 rows)
GRP = 512                # psum bank free size (f32)
# dst phases (col0, width): wide early phases amortize LDWEIGHTS (one
# stationary load serves all groups of a phase per v); the 512-col
# final phase shrinks the post-stream tail to one matmul + one fold +
# one 65 KB store. Groups (width/512) sum to the 8 PSUM banks: 4+3+1.
_PH = int(os.environ.get("KERNEL_PHASEPLAN", "3"))
if _PH == 2:
    PHASES = [(0, 3584), (3584, 512)]
elif _PH == 4:
    PHASES = [(0, 2048), (2048, 1536), (3584, 256), (3840, 256)]
else:
    PHASES = [(0, 2048), (2048, 1536), (3584, 512)]

# Aᵀ chunk plan: (phase, v0, nv) in issue/consumption order. Small
# leading chunks let the rings start transferring while desc-gen for
# the rest proceeds; small final chunks trim the tail matmul lag.
def _spans(sizes):
    out, v0 = [], 0
    for s in sizes:
        out.append((v0, s)); v0 += s
    return out

if _PH == 2:
    _QSPANS = {
        0: _spans([1, 1, 2, 2, 2, 2, 2, 2, 2]),
        1: _spans([4, 4, 4, 2, 1, 1]),
    }
elif _PH == 4:
    _QSPANS = {
        0: _spans([1, 1, 2, 4, 4, 4]),   # 512 KB per v-unit
        1: _spans([4, 4, 4, 4]),         # 384 KB per v-unit
        2: _spans([8, 8]),               # 64 KB per v-unit
        3: _spans([8, 4, 2, 1, 1]),      # 64 KB per v-unit
    }
else:
    _QSPANS = {
        0: _spans([1, 1, 2, 4, 4, 4]),   # 512 KB per v-unit
        1: _spans([4, 4, 4, 4]),         # 384 KB per v-unit
        # last v-block alone in the final 128 KB chunk: only ONE matmul
        # (not two) trails the final arrival + completion-sem latency
        2: _spans([4, 4, 4, 3, 1]),      # 128 KB per v-unit
    }
CHUNKS = [(ph, v0, nv) for ph in range(len(PHASES))
          for (v0, nv) in _QSPANS[ph]]

LAST_EXEC_NS = None


# ---------------------------------------------------------------------------
# axon NTFF profile hook shim (the agent image's antenv lacks axon_hooks)
# ---------------------------------------------------------------------------
def _install_profile_shim():
    if "antenv.axon_hooks" in sys.modules:
        return
    try:
        import antenv

        mod = types.ModuleType("antenv.axon_hooks")
        mod._hook = None
        mod.set_axon_ntff_profile_hook = lambda h: setattr(mod, "_hook", h)
        mod.get_axon_ntff_profile_hook = lambda: mod._hook
        sys.modules["antenv.axon_hooks"] = mod
        antenv.axon_hooks = mod
        from trn_agent_boot.trn_boot import _ntff_profile_via_ctypes

        mod.set_axon_ntff_profile_hook(
            _ntff_profile_via_ctypes("/opt/axon/libaxon_pjrt.so")
        )
    except Exception:
        pass


# ---------------------------------------------------------------------------
# Walrus in this container rejects >1 sync-wait per instruction. Split any
# multi-wait instruction into single-wait NoOps on the same engine.
# ---------------------------------------------------------------------------
def _split_multiwaits(nc, mybir):
    cnt = [0]
    for f in nc.m.functions:
        for bb in f.blocks:
            new_list = []
            for ins in bb.instructions:
                si = ins.sync_info
                if si is not None and si.on_wait and len(si.on_wait) > 1:
                    waits = list(si.on_wait)
                    for w in waits[:-1]:
                        cnt[0] += 1
                        nop = mybir.InstNoOp(name=f"I-waitsplit-{cnt[0]}")
                        nop.engine = ins.engine
                        nop.sync_info = mybir.SyncInfo(on_wait=[w], on_update=[])
                        try:
                            nc.register_instruction(nop, overwrite=True)
                        except Exception:
                            pass
                        new_list.append(nop)
                    ins.sync_info = mybir.SyncInfo(
                        on_wait=[waits[-1]], on_update=list(si.on_update)
                    )
                new_list.append(ins)
            bb.instructions = new_list


# ---------------------------------------------------------------------------
# Legalization splits every InstMatmult into Ldweights+Matmult, reloading
# the stationary even when consecutive matmuls share it. Drop the redundant
# reloads (PE array state persists); preserve any sync via a PE NoOp.
# ---------------------------------------------------------------------------
def _dedup_ldweights(nc, mybir):
    dropped = [0]
    for f in nc.m.functions:
        for bb in f.blocks:
            last_sig = None
            new_list = []
            for ins in bb.instructions:
                if isinstance(ins, mybir.InstLdweights):
                    sig = repr(ins.ins[0])
                    if sig == last_sig:
                        si = ins.sync_info
                        dropped[0] += 1
                        if si is not None and (si.on_wait or si.on_update):
                            nop = mybir.InstNoOp(
                                name=f"I-ldwdedup-{dropped[0]}")
                            nop.engine = ins.engine
                            nop.sync_info = si
                            try:
                                nc.register_instruction(nop, overwrite=True)
                            except Exception:
                                pass
                            new_list.append(nop)
                        continue
                    last_sig = sig
                elif isinstance(ins, mybir.InstMatmult):
                    pass  # uses the loaded array, does not clobber it
                elif getattr(ins, "engine", None) == mybir.EngineType.PE:
                    last_sig = None  # unknown PE instruction: be safe
                new_list.append(ins)
            bb.instructions = new_list
    return dropped[0]


# ---------------------------------------------------------------------------
# Device program
# ---------------------------------------------------------------------------
SWI = bool(int(os.environ.get("KERNEL_SWI", "1")))  # DoubleRowSwInterleave
NWARM = int(os.environ.get("KERNEL_NWARM", "20"))


def _build_program():
    import concourse.mybir as mybir
    import concourse.tile as tile
    from concourse import bacc

    nc = bacc.Bacc("TRN2", target_bir_lowering=False, debug=False,
                   num_devices=NCORES, num_swdge_queues=4,
                   dynamic_dma_scratch_size=32768)
    # Aᵀ fp8 byte chunks.  Chunk (ph, v0, nv) with phase (col0, width)
    # holds, p-major:
    #   at_c[p, ((v-v0)*2 + t)*width + n] = A[256*v + 128*t + p, col0 + n]
    # (A includes the (1+alpha) diagonal).
    at_d = [
        nc.dram_tensor(f"at{c}", [P, nv * 2 * PHASES[ph][1]],
                       mybir.dt.uint8, kind="ExternalInput")
        for c, (ph, v0, nv) in enumerate(CHUNKS)
    ]
    # stationary x fp8 bytes: [p, (q, t, s, c)] with s = {hi, w}
    xs_d = nc.dram_tensor("xs", [P, NPAIR * 2 * 2 * C], mybir.dt.uint8,
                          kind="ExternalInput")
    out_d = nc.dram_tensor("out", [C, N], mybir.dt.bfloat16,
                           kind="ExternalOutput")

    fp8 = mybir.dt.float8e4
    ring_plan = {}

    with tile.TileContext(nc) as tc:
        with tc.tile_pool(name="sbuf", bufs=1) as pool, \
             tc.tile_pool(name="psum", bufs=1, space="PSUM") as ppool:
            xs_sb = pool.tile([P, NPAIR * 2 * 2 * C], mybir.dt.uint8,
                              tag="xs")
            o_sb = pool.tile([C, N], mybir.dt.bfloat16, tag="o")
            GW = []  # per-bank group width
            for (col0, width) in PHASES:
                gw = min(GRP, width)
                GW += [gw] * (width // gw)
            NBANK = len(GW)
            t_sb = [pool.tile([C, GW[g]], mybir.dt.bfloat16, tag=f"t{g}",
                              name=f"t{g}") for g in range(NBANK)]
            ps = [ppool.tile([P, GW[g]], mybir.dt.float32, tag=f"ps{g}",
                             name=f"ps{g}") for g in range(NBANK)]
            # one tile per chunk: slices of a shared tile would create
            # false write-after-read deps in the tile tracker and stall
            # the Q7 desc-gen at phase boundaries
            at_sb = [pool.tile([P, nv * 2 * PHASES[ph][1]], mybir.dt.uint8,
                               tag=f"at{c}", name=f"at{c}")
                     for c, (ph, v0, nv) in enumerate(CHUNKS)]
            # map (phase, v) -> (chunk tile, local v)
            v_map = {}
            for c, (ph, v0, nv) in enumerate(CHUNKS):
                for dv in range(nv):
                    v_map[(ph, v0 + dv)] = (c, dv)

            def swdge(inst, ring):
                ring_plan[inst.ins.name] = ring
                return inst

            # xs first (gates the warmup matmuls), then the Aᵀ chunks.
            # Ring = (SWDGE instruction index) % 4, which matches the
            # DMASW lane-locking exactly (lane = idx % 8, lane -> ring
            # idx % 4), so the post-compile patch never has to override.
            sw_idx = [0]

            def swdge_auto(inst):
                ring_plan[inst.ins.name] = sw_idx[0] % 4
                sw_idx[0] += 1
                return inst

            # NOTE: offloading chunks/stores to the HWDGE rings was tried
            # and measured ~16 GB/s while the SWDGE stream saturates the
            # 16 shared SDMA engines -- never do it.
            swdge_auto(nc.gpsimd.dma_start(out=xs_sb[:], in_=xs_d.ap()))
            for c, (ph, v0, nv) in enumerate(CHUNKS):
                swdge_auto(nc.gpsimd.dma_start(
                    out=at_sb[c][:], in_=at_d[c].ap()))

            # stationary views: [128, 2, 128] fp8 per pair v (hi|w columns)
            xs3 = xs_sb[:].rearrange("p (v t sc) -> p v t sc",
                                     v=NPAIR, t=2)

            # PE p-state warm-up: garbage matmuls (into ps[0], discarded by
            # the first real start=True matmul) so the tensor engine is at
            # full clock when the Aᵀ stream lands.
            wmv = xs_sb[:].rearrange("p (t n) -> p t n", t=2).bitcast(fp8)
            for i in range(NWARM):
                nc.tensor.matmul(
                    ps[0][:],
                    xs3[:, 0, :, :].bitcast(fp8),
                    wmv[:, :, 0:GRP],
                    start=(i == 0),
                    stop=(i == NWARM - 1),
                    perf_mode=(
                        mybir.MatmulPerfMode.DoubleRowSwInterleave
                        if SWI else mybir.MatmulPerfMode.DoubleRow),
                    skip_group_check=True,
                )

            PM = (mybir.MatmulPerfMode.DoubleRowSwInterleave if SWI
                  else mybir.MatmulPerfMode.DoubleRow)
            bank0 = 0
            for ph, (col0, width) in enumerate(PHASES):
                gw = min(GRP, width)
                ngrp = width // gw
                for v in range(NPAIR):
                    c, dv = v_map[(ph, v)]
                    nvc = CHUNKS[c][2]
                    at4 = at_sb[c][:].rearrange("p (v t n) -> p v t n",
                                                v=nvc, t=2).bitcast(fp8)
                    lhsT = xs3[:, v, :, :].bitcast(fp8)
                    for g in range(ngrp):
                        mv = at4[:, dv, :, g * gw:(g + 1) * gw]
                        nc.tensor.matmul(
                            ps[bank0 + g][:], lhsT, mv,
                            start=(v == 0),
                            stop=(v == NPAIR - 1),
                            perf_mode=PM,
                        )
                # phase ph done: fold hi+w partitions, store on the SWDGE
                # rings. DVE can only read ONE input from PSUM, so ACT
                # copies the w rows to SBUF (pipelines with DVE across
                # groups). All stores ride SWDGE: HWDGE measured ~16 GB/s
                # under stream load and only ~34 GB/s even on idle engines,
                # and splitting the final fold/store into 256-col halves
                # lost 0.4us to per-instruction semaphore latency.
                for g in range(ngrp):
                    gi = bank0 + g
                    lo = col0 + g * gw
                    nc.scalar.copy(out=t_sb[gi][:], in_=ps[gi][C:2 * C, :])
                    nc.vector.tensor_add(
                        out=o_sb[:, lo:lo + gw],
                        in0=ps[gi][0:C, :],
                        in1=t_sb[gi][:],
                    )
                swdge_auto(nc.gpsimd.dma_start(
                    out=out_d.ap()[:, col0:col0 + width],
                    in_=o_sb[:, col0:col0 + width],
                ))
                bank0 += ngrp

    nc.compile()
    # Spread SWDGE dma_starts across the 4 queue rings (see baseline notes:
    # DMASW sem lanes are handed out round-robin over the final instruction
    # order and each lane locks to one ring, so walk the compiled order and
    # keep lanes consistent, preferring the planned ring at each lane's
    # first use).
    lane_ring = {}
    idx = 0
    for f in nc.m.functions:
        for bb in f.blocks:
            for ins in bb.instructions:
                if ins.name not in ring_plan:
                    continue
                lane = idx % 8
                idx += 1
                ring = lane_ring.setdefault(lane, ring_plan[ins.name])
                if ring:
                    ins.queue = f"qPoolDynamic{ring}"
    if bool(int(os.environ.get("KERNEL_DEDUP_LDW", "1"))):
        _dedup_ldweights(nc, mybir)
    _split_multiwaits(nc, mybir)
    return nc


_PROGRAM = None
_PROGRAM_ALPHA = None


def _get_program(alpha_v):
    global _PROGRAM, _PROGRAM_ALPHA
    if _PROGRAM is None:
        _PROGRAM = _build_program()
        _PROGRAM_ALPHA = alpha_v
    return _PROGRAM


# ---------------------------------------------------------------------------
# Host glue
# ---------------------------------------------------------------------------
def _fp8_lut():
    import ml_dtypes

    return np.arange(K + 1).astype(ml_dtypes.float8_e4m3fn).view(np.uint8)


_LUT = None


def _prep_at_chunks(edge_b, alpha_v):
    """edge_b [N, K] int32 -> list of Aᵀ fp8 chunk arrays (device layout).

    Aᵀ[src, dst] = |{k: edge[dst, k] == src}| + (1+alpha)*[src == dst].
    For integer (1+alpha) with max entry <= 16 this is exact in e4m3 via
    the count LUT.
    """
    global _LUT
    if _LUT is None:
        _LUT = _fp8_lut()
    src = edge_b.astype(np.int64)                       # [N dst, K]
    flat = (src * N + np.arange(N, dtype=np.int64)[:, None]).ravel()
    cnt = np.bincount(flat, minlength=N * N)            # Aᵀ[src, dst] counts
    a1 = float(1.0 + alpha_v)
    diag = np.arange(N, dtype=np.int64) * (N + 1)
    if a1 == round(a1) and 0 <= a1 <= 16:
        cnt[diag] += int(round(a1))
        assert cnt[diag].max() <= K, "diagonal count overflow for e4m3"
        at = _LUT[cnt]                                  # uint8 fp8 bytes
    else:
        import ml_dtypes

        vals = cnt.astype(np.float32)
        vals[diag] += np.float32(a1)
        q = vals.astype(ml_dtypes.float8_e4m3fn)
        assert np.array_equal(q.astype(np.float32), vals), \
            "1+alpha not exactly representable in e4m3"
        at = q.view(np.uint8)
    at = at.reshape(N, N)
    chunks = []
    for (ph, v0, nv) in CHUNKS:
        col0, width = PHASES[ph]
        # rows 256v+128t+p for v in [v0, v0+nv), cols col0 + n
        blk = at[256 * v0:256 * (v0 + nv), col0:col0 + width]
        b4 = blk.reshape(nv, 2, P, width)               # (v, t, p, n)
        b4 = b4.transpose(2, 0, 1, 3)                   # (p, v, t, n)
        chunks.append(np.ascontiguousarray(
            b4.reshape(P, nv * 2 * width)))
    return chunks


def _prep_xs(xt_b):
    """xt_b [N, C] f32 node-major -> stationary fp8 bytes [128, v*t*s*C]."""
    import ml_dtypes

    hi = xt_b.astype(ml_dtypes.float8_e4m3fn)
    w = (xt_b - hi.astype(np.float32)).astype(ml_dtypes.float8_e4m3fn)
    hw = np.stack([hi.view(np.uint8), w.view(np.uint8)], axis=1)  # [N, s, C]
    hw = hw.reshape(NPAIR, 2, P, 2, C)                  # (v, t, p, s, c)
    hw = hw.transpose(2, 0, 1, 3, 4)                    # (p, v, t, s, c)
    out = np.ascontiguousarray(hw.reshape(P, NPAIR * 2 * 2 * C))
    if SWI:
        # DoubleRowSwInterleave weight layout: flat cols per pair v are
        # [A127 B127 A126 B126 ... A0 B0] where A/B are the two k-tiles
        # and the logical column index is reversed.
        w4 = out.reshape(P, NPAIR, 2, 2 * C)            # (p, v, t, m)
        wr = w4[:, :, :, ::-1]                          # m -> reversed j
        wi = np.transpose(wr, (0, 1, 3, 2))             # (p, v, j, t)
        out = np.ascontiguousarray(wi.reshape(P, NPAIR * 2 * 2 * C))
    return out


def kernel(x, edge_index, alpha):
    global LAST_EXEC_NS
    _install_profile_shim()
    from concourse import bass_utils

    x = np.asarray(x)
    edge_index = np.asarray(edge_index)
    alpha_v = np.float32(np.asarray(alpha))

    nc = _get_program(float(alpha_v))

    in_maps = []
    for b in range(B):
        xt = np.ascontiguousarray(x[b, :, :, 0].T)      # [N, C]
        m = {"xs": _prep_xs(xt)}
        for c, arr in enumerate(_prep_at_chunks(edge_index[b], alpha_v)):
            m[f"at{c}"] = arr
        in_maps.append(m)

    trace = bool(int(os.environ.get("KERNEL_PROFILE", "0")))
    res = bass_utils.run_bass_kernel_spmd(
        nc, in_maps, core_ids=list(range(NCORES)), trace=trace
    )
    LAST_EXEC_NS = res.exec_time_ns

    out = np.empty((B, C, N, 1), dtype=np.float32)
    for b in range(B):
        out[b, :, :, 0] = res.results[b]["out"].astype(np.float32)
    return out
